# revision 7
# baseline (speedup 1.0000x reference)
"""GATv2 layer (N=1024, IN=OUT=128, H=4, D=32) on 8 Trainium2 NeuronCores.

Sharding: row-block of the output/adjacency (128 rows of i per core);
node features (pre-transposed h^T) and the projection weights are replicated.

Math per core (rows i of this core's block), with leakyrelu(x) = x - 0.8*min(x,0)
= 0.6*x + 0.4*|x| and sl[i,h] = a.Wlh[i,h,:] (cancels in the softmax over j),
sr[j,h] = a.Wrh[j,h,:]:

  e[i,j,h] = c*sr[j,h] + m_i[:,j] @ blockdiag(s*a)  - 100*(1-adj[i,j])

where per i-row either m_i = min(Wrh^T + Wlh_i, 0) produced on DVE
(tensor_scalar add+min, c=1, s=-0.8) or m_i = |Wrh^T + Wlh_i| produced on ACT
(Abs with per-partition bias, c=0.6, s=0.4).  Scores are O(3) so no
max-subtraction is needed, and the -100 mask term underflows exp to exact 0.
The d-reduction runs on the PE with m_i as the (FWL bf16) weights:
  bank_jt[j_local, 4*i+h] += m_i[:, jt].T @ blockdiag(s*a)
Two more accumulating matmuls per bank add c*sr (rank-4, with the per-column c
baked into the rhs) and the mask (via an identity-expansion rhs).  ACT
exponentiates banks into bf16 w^T tiles [j, (i,h)] and the PE aggregates
agg[i,(h,d)] plus the softmax denominators in one pass using V extended with a
ones-column.  LayerNorm: bn_stats + rstd = exp(-0.5*ln(var+eps)).
"""
import numpy as np
import ml_dtypes

import concourse.bacc as bacc
import concourse.tile as tile
from concourse import mybir
from concourse.bass_utils import run_bass_kernel_spmd

N = 1024
IN_DIM = 128
OUT_DIM = 128
H = 4
D = 32
NCORES = 8
BLK = N // NCORES  # 128 rows of i per core
NJT = 8            # j tiles of 128
F32 = mybir.dt.float32
BF16 = mybir.dt.bfloat16
I32 = mybir.dt.int32
AF = mybir.ActivationFunctionType
ALU = mybir.AluOpType

ACT_EVERY = 4  # every ACT_EVERY-th |pair| tile is produced on ACT


def _on_act(i):
    return i % ACT_EVERY == ACT_EVERY - 1


def build_program():
    nc = bacc.Bacc(trn_type="TRN2", target_bir_lowering=False, debug=False,
                   num_devices=NCORES)

    def din(name, shape, dt):
        return nc.dram_tensor(name, shape, dt, kind="ExternalInput").ap()

    hT_d = din("hT", [IN_DIM, N], F32)               # h^T (host-transposed)
    hblkT_d = din("hblkT", [IN_DIM, BLK], F32)       # this core's rows of h, ^T
    adj_d = din("adjrow", [BLK, N], I32)             # this core's adj rows
    wl_d = din("W_l", [IN_DIM, OUT_DIM], F32)
    wr_d = din("W_r", [IN_DIM, OUT_DIM], F32)
    wv_d = din("W_v", [IN_DIM, OUT_DIM], F32)
    adve_d = din("Adve", [128, H], BF16)             # blockdiag(-0.8*a)
    aact_d = din("Aact", [128, H], BF16)             # blockdiag(+0.4*a)
    a1_d = din("A1", [128, H], BF16)                 # blockdiag(a)
    repc_d = din("rep_c", [H, H * BLK], BF16)        # c_i * I4 per i-column
    i4rep_d = din("I4rep", [BLK, H * BLK], BF16)     # repeat(I128, 4, axis=1)
    gbc_d = din("gbc", [BLK, OUT_DIM], F32)          # ln_g broadcast
    bbc_d = din("bbc", [BLK, OUT_DIM], F32)          # ln_b broadcast
    y_d = nc.dram_tensor("y", [BLK, OUT_DIM], F32, kind="ExternalOutput").ap()

    with tile.TileContext(nc) as tc:
        with tc.tile_pool(name="keep", bufs=1) as keep, \
             tc.tile_pool(name="small", bufs=4) as small:
            # --- loads on the stage-1 critical path first ---
            hT_sb = keep.tile([IN_DIM, N], F32)
            nc.sync.dma_start(out=hT_sb, in_=hT_d)
            wr_sb = keep.tile([IN_DIM, OUT_DIM], F32)
            nc.sync.dma_start(out=wr_sb, in_=wr_d)
            hblkT_sb = keep.tile([IN_DIM, BLK], F32)
            nc.sync.dma_start(out=hblkT_sb, in_=hblkT_d)
            wl_sb = keep.tile([IN_DIM, OUT_DIM], F32)
            nc.sync.dma_start(out=wl_sb, in_=wl_d)
            adve_sb = keep.tile([128, H], BF16)
            nc.sync.dma_start(out=adve_sb, in_=adve_d)
            aact_sb = keep.tile([128, H], BF16)
            nc.sync.dma_start(out=aact_sb, in_=aact_d)
            adj_sb = keep.tile([BLK, N], I32)
            nc.sync.dma_start(out=adj_sb, in_=adj_d)
            wv_sb = keep.tile([IN_DIM, OUT_DIM], F32)
            nc.sync.dma_start(out=wv_sb, in_=wv_d)
            a1_sb = keep.tile([128, H], BF16)
            nc.sync.dma_start(out=a1_sb, in_=a1_d)
            repc_sb = keep.tile([H, H * BLK], BF16)
            nc.sync.dma_start(out=repc_sb, in_=repc_d)
            i4rep_sb = keep.tile([BLK, H * BLK], BF16)
            nc.sync.dma_start(out=i4rep_sb, in_=i4rep_d)
            gbc_sb = keep.tile([BLK, OUT_DIM], F32)
            nc.sync.dma_start(out=gbc_sb, in_=gbc_d)
            bbc_sb = keep.tile([BLK, OUT_DIM], F32)
            nc.sync.dma_start(out=bbc_sb, in_=bbc_d)

            wrhT_sb = keep.tile([128, N], BF16)       # (h@W_r)^T  [hd, j]
            wlhT_sb = keep.tile([128, BLK], F32)      # (hblk@W_l)^T [hd, i]
            vext_sb = keep.tile([128, NJT * (D + 1) * H], BF16)  # V + ones cols
            srT_sb = keep.tile([H, N], BF16)          # sr^T [h, j]
            maskb_sb = keep.tile([BLK, N], BF16)      # (adj-1)*100
            wT_sb = keep.tile([128, NJT * H * BLK], BF16)  # exp scores [j,(i,h)]
            agg_sb = keep.tile([BLK, OUT_DIM], F32)

            with tc.tile_pool(name="ps0", bufs=2, space="PSUM") as ps0:
                # WrhT = W_r^T @ h^T  -> bf16 (gates stage 1)
                for half in range(2):
                    big = ps0.tile([128, 512], F32, tag="big")
                    nc.tensor.matmul(big, wr_sb, hT_sb[:, half * 512:(half + 1) * 512],
                                     start=True, stop=True)
                    nc.vector.tensor_copy(wrhT_sb[:, half * 512:(half + 1) * 512], big)
                # WlhT (this block), kept f32 for scalar/bias use
                wp = ps0.tile([128, 128], F32, tag="tp")
                nc.tensor.matmul(wp, wl_sb, hblkT_sb, start=True, stop=True)
                nc.vector.tensor_copy(wlhT_sb, wp)
                # mask = (adj-1)*100 in bf16 (cheap, DVE, before the absp stream)
                nc.vector.tensor_scalar(maskb_sb, adj_sb, 1.0, 100.0,
                                        ALU.subtract, ALU.mult)
                # V projection + Vext (ACT evacuates to overlap with DVE work)
                nc.vector.memset(vext_sb, 1.0)
                for jt in range(NJT):
                    vp = ps0.tile([128, 128], F32, tag="tp")
                    nc.tensor.matmul(vp, hT_sb[:, jt * 128:(jt + 1) * 128],
                                     wv_sb, start=True, stop=True)
                    base = jt * (D + 1) * H
                    for hh in range(H):
                        nc.scalar.copy(
                            vext_sb[:, base + hh * (D + 1):
                                    base + hh * (D + 1) + D],
                            vp[:, hh * D:(hh + 1) * D])
                # srT = a^T . WrhT per head
                for half in range(2):
                    sp = ps0.tile([H, 512], F32, tag="sr")
                    nc.tensor.matmul(sp, a1_sb,
                                     wrhT_sb[:, half * 512:(half + 1) * 512],
                                     start=True, stop=True)
                    nc.vector.tensor_copy(
                        srT_sb[:, half * 512:(half + 1) * 512], sp)

            # ------------- stage 1: pairwise scores -------------
            with tc.tile_pool(name="ps1", bufs=NJT, space="PSUM") as ps1, \
                 tc.tile_pool(name="abs", bufs=6) as absp_pool:
                banks = [ps1.tile([128, H * BLK], F32, name=f"bank{jt}", tag="bank")
                         for jt in range(NJT)]
                for jt in range(NJT):
                    # sr term (rank-4) opens each bank's accumulation group
                    nc.tensor.matmul(banks[jt], srT_sb[:, jt * 128:(jt + 1) * 128],
                                     repc_sb, start=True, stop=False,
                                     skip_group_check=True)
                    nc.tensor.matmul(banks[jt], maskb_sb[:, jt * 128:(jt + 1) * 128],
                                     i4rep_sb, start=False, stop=False,
                                     skip_group_check=True)
                for i in range(BLK):
                    absp = absp_pool.tile([128, N], BF16, tag="absp")
                    if _on_act(i):
                        # |WrhT + wl_i|
                        nc.scalar.activation(absp, wrhT_sb, AF.Abs,
                                             bias=wlhT_sb[:, i:i + 1], scale=1.0)
                        arhs = aact_sb
                    else:
                        # min(WrhT + wl_i, 0)
                        nc.vector.tensor_scalar(absp, wrhT_sb,
                                                wlhT_sb[:, i:i + 1],
                                                0.0, ALU.add, ALU.min)
                        arhs = adve_sb
                    for jt in range(NJT):
                        nc.tensor.matmul(banks[jt][:, H * i:H * i + H],
                                         absp[:, jt * 128:(jt + 1) * 128], arhs,
                                         start=False, stop=(i == BLK - 1),
                                         skip_group_check=True)
                # exp -> w^T bf16 (bank_jt frees as soon as its exp is done)
                for jt in range(NJT):
                    nc.scalar.activation(wT_sb[:, jt * 512:(jt + 1) * 512],
                                         banks[jt], AF.Exp)

            # ------------- stage 3: aggregate, pipelined with exp -------------
            with tc.tile_pool(name="ps3", bufs=4, space="PSUM") as ps3:
                accs = [ps3.tile([BLK, D + 1], F32, name=f"acc{hh}", tag="acc") for hh in range(H)]
                for jt in range(NJT):
                    for hh in range(H):
                        lhsT = wT_sb[:, jt * 512 + hh:(jt + 1) * 512: H].opt()
                        rhs = vext_sb[:, jt * (D + 1) * H + hh * (D + 1):
                                      jt * (D + 1) * H + (hh + 1) * (D + 1)]
                        nc.tensor.matmul(accs[hh], lhsT, rhs,
                                         start=(jt == 0), stop=(jt == NJT - 1),
                                         skip_group_check=True)
                for hh in range(H):
                    rinv = small.tile([BLK, 1], F32, tag="rinv")
                    nc.vector.reciprocal(rinv, accs[hh][:, D:D + 1])
                    nc.vector.tensor_scalar_mul(
                        agg_sb[:, hh * D:(hh + 1) * D], accs[hh][:, 0:D], rinv)

            # ---------------- stage 4: LayerNorm + ReLU ----------------
            stats = small.tile([BLK, 6], F32, tag="stats")
            nc.vector.bn_stats(out=stats, in_=agg_sb)
            mv = small.tile([BLK, 2], F32, tag="mv")
            nc.vector.bn_aggr(out=mv, in_=stats)
            cent = keep.tile([BLK, OUT_DIM], F32)
            nc.vector.tensor_scalar_sub(cent, agg_sb, mv[:, 0:1])
            epsb = small.tile([BLK, 1], F32, tag="epsb")
            nc.vector.memset(epsb, 1e-5)
            lnv = small.tile([BLK, 1], F32, tag="lnv")
            nc.scalar.activation(lnv, mv[:, 1:2], AF.Ln, bias=epsb, scale=1.0)
            rstd = small.tile([BLK, 1], F32, tag="rstd")
            nc.scalar.activation(rstd, lnv, AF.Exp, scale=-0.5)
            yt = keep.tile([BLK, OUT_DIM], F32)
            nc.vector.tensor_scalar_mul(yt, cent, rstd)
            nc.vector.tensor_tensor(yt, yt, gbc_sb, ALU.mult)
            nc.vector.tensor_tensor(yt, yt, bbc_sb, ALU.add)
            nc.vector.tensor_scalar_max(yt, yt, 0.0)
            nc.sync.dma_start(out=y_d, in_=yt)

    nc.compile()
    return nc


_NC = None


def _get_program():
    global _NC
    if _NC is None:
        _NC = build_program()
    return _NC


def _consts(a, ln_g, ln_b):
    bf = ml_dtypes.bfloat16
    a = np.asarray(a, np.float32)
    Adve = np.zeros((128, H), np.float32)
    Aact = np.zeros((128, H), np.float32)
    A1 = np.zeros((128, H), np.float32)
    for hh in range(H):
        Adve[hh * D:(hh + 1) * D, hh] = -0.8 * a
        Aact[hh * D:(hh + 1) * D, hh] = 0.4 * a
        A1[hh * D:(hh + 1) * D, hh] = a
    # per-i sr coefficient: 0.6 for ACT(|.|) rows, 1.0 for DVE(min) rows
    rep_c = np.zeros((H, H * BLK), np.float32)
    for i in range(BLK):
        c = 0.6 if _on_act(i) else 1.0
        rep_c[:, H * i:H * i + H] = c * np.eye(H, dtype=np.float32)
    I4rep = np.repeat(np.eye(BLK, dtype=np.float32), H, axis=1)
    return {
        "Adve": Adve.astype(bf), "Aact": Aact.astype(bf), "A1": A1.astype(bf),
        "rep_c": rep_c.astype(bf), "I4rep": I4rep.astype(bf),
        "gbc": np.tile(np.asarray(ln_g, np.float32)[None, :], (BLK, 1)),
        "bbc": np.tile(np.asarray(ln_b, np.float32)[None, :], (BLK, 1)),
    }


def kernel(h, adj, W_l, W_r, W_v, a, ln_g, ln_b, _trace=False, _tmpdir=None):
    nc = _get_program()
    h = np.asarray(h, np.float32)
    hT = np.ascontiguousarray(h.T)
    adj = np.ascontiguousarray(np.asarray(adj, np.int32))
    consts = _consts(a, ln_g, ln_b)
    base = {
        "hT": hT,
        "W_l": np.ascontiguousarray(np.asarray(W_l, np.float32)),
        "W_r": np.ascontiguousarray(np.asarray(W_r, np.float32)),
        "W_v": np.ascontiguousarray(np.asarray(W_v, np.float32)),
        **consts,
    }
    in_maps = []
    for c in range(NCORES):
        m = dict(base)
        m["hblkT"] = np.ascontiguousarray(hT[:, c * BLK:(c + 1) * BLK])
        m["adjrow"] = adj[c * BLK:(c + 1) * BLK]
        in_maps.append(m)
    kw = {}
    if _trace:
        kw = dict(trace=True, tmpdir=_tmpdir)
    res = run_bass_kernel_spmd(nc, in_maps, list(range(NCORES)), **kw)
    y = np.concatenate([res.results[c]["y"] for c in range(NCORES)], axis=0)
    if _trace:
        return y, res
    return y


# revision 9
# speedup vs baseline: 1.0229x; 1.0229x over previous
"""GATv2 layer (N=1024, IN=OUT=128, H=4, D=32) on 8 Trainium2 NeuronCores.

Sharding: row-block of the output/adjacency (128 rows of i per core);
node features (pre-transposed h^T) and the projection weights are replicated.

Math per core (rows i of this core's block), with leakyrelu(x) = x - 0.8*min(x,0)
= 0.6*x + 0.4*|x| and sl[i,h] = a.Wlh[i,h,:] (cancels in the softmax over j),
sr[j,h] = a.Wrh[j,h,:]:

  e[i,j,h] = c*sr[j,h] + m_i[:,j] @ blockdiag(s*a)  - 100*(1-adj[i,j])

where per i-row either m_i = min(Wrh^T + Wlh_i, 0) produced on DVE
(tensor_scalar add+min, c=1, s=-0.8) or m_i = |Wrh^T + Wlh_i| produced on ACT
(Abs with per-partition bias, c=0.6, s=0.4).  Scores are O(3) so no
max-subtraction is needed, and the -100 mask term underflows exp to exact 0.
The d-reduction runs on the PE with m_i as the (FWL bf16) weights:
  bank_jt[j_local, 4*i+h] += m_i[:, jt].T @ blockdiag(s*a)
Two more accumulating matmuls per bank add c*sr (rank-4, with the per-column c
baked into the rhs) and the mask (via an identity-expansion rhs).  ACT
exponentiates banks into bf16 w^T tiles [j, (i,h)] and the PE aggregates
agg[i,(h,d)] plus the softmax denominators in one pass using V extended with a
ones-column.  LayerNorm: bn_stats + rstd = exp(-0.5*ln(var+eps)).
"""
import numpy as np
import ml_dtypes

import concourse.bacc as bacc
import concourse.tile as tile
from concourse import mybir
from concourse.bass_utils import run_bass_kernel_spmd

N = 1024
IN_DIM = 128
OUT_DIM = 128
H = 4
D = 32
NCORES = 8
BLK = N // NCORES  # 128 rows of i per core
NJT = 8            # j tiles of 128
F32 = mybir.dt.float32
BF16 = mybir.dt.bfloat16
I32 = mybir.dt.int32
AF = mybir.ActivationFunctionType
ALU = mybir.AluOpType

ACT_EVERY = 4  # every ACT_EVERY-th |pair| tile is produced on ACT


def _on_act(i):
    return i % ACT_EVERY == ACT_EVERY - 1


def build_program():
    nc = bacc.Bacc(trn_type="TRN2", target_bir_lowering=False, debug=False,
                   num_devices=NCORES)

    def din(name, shape, dt):
        return nc.dram_tensor(name, shape, dt, kind="ExternalInput").ap()

    hT_d = din("hT", [IN_DIM, N], F32)               # h^T (host-transposed)
    hblkT_d = din("hblkT", [IN_DIM, BLK], F32)       # this core's rows of h, ^T
    adj_d = din("adjrow", [BLK, N], I32)             # this core's adj rows
    wl_d = din("W_l", [IN_DIM, OUT_DIM], F32)
    wr_d = din("W_r", [IN_DIM, OUT_DIM], F32)
    wv_d = din("W_v", [IN_DIM, OUT_DIM], F32)
    adve_d = din("Adve", [128, H], BF16)             # blockdiag(-0.8*a)
    aact_d = din("Aact", [128, H], BF16)             # blockdiag(+0.4*a)
    a1_d = din("A1", [128, H], BF16)                 # blockdiag(a)
    repc_d = din("rep_c", [H, H * BLK], BF16)        # c_i * I4 per i-column
    i4rep_d = din("I4rep", [BLK, H * BLK], BF16)     # repeat(I128, 4, axis=1)
    gbc_d = din("gbc", [BLK, OUT_DIM], F32)          # ln_g broadcast
    bbc_d = din("bbc", [BLK, OUT_DIM], F32)          # ln_b broadcast
    y_d = nc.dram_tensor("y", [BLK, OUT_DIM], F32, kind="ExternalOutput").ap()

    with tile.TileContext(nc) as tc:
        with tc.tile_pool(name="keep", bufs=1) as keep, \
             tc.tile_pool(name="small", bufs=4) as small:
            # --- loads on the stage-1 critical path first ---
            hT_sb = keep.tile([IN_DIM, N], F32)
            nc.sync.dma_start(out=hT_sb, in_=hT_d)
            wr_sb = keep.tile([IN_DIM, OUT_DIM], F32)
            nc.sync.dma_start(out=wr_sb, in_=wr_d)
            hblkT_sb = keep.tile([IN_DIM, BLK], F32)
            nc.sync.dma_start(out=hblkT_sb, in_=hblkT_d)
            wl_sb = keep.tile([IN_DIM, OUT_DIM], F32)
            nc.sync.dma_start(out=wl_sb, in_=wl_d)
            adve_sb = keep.tile([128, H], BF16)
            nc.sync.dma_start(out=adve_sb, in_=adve_d)
            aact_sb = keep.tile([128, H], BF16)
            nc.sync.dma_start(out=aact_sb, in_=aact_d)
            adj_sb = keep.tile([BLK, N], I32)
            nc.sync.dma_start(out=adj_sb, in_=adj_d)
            wv_sb = keep.tile([IN_DIM, OUT_DIM], F32)
            nc.sync.dma_start(out=wv_sb, in_=wv_d)
            a1_sb = keep.tile([128, H], BF16)
            nc.sync.dma_start(out=a1_sb, in_=a1_d)
            repc_sb = keep.tile([H, H * BLK], BF16)
            nc.sync.dma_start(out=repc_sb, in_=repc_d)
            i4rep_sb = keep.tile([BLK, H * BLK], BF16)
            nc.sync.dma_start(out=i4rep_sb, in_=i4rep_d)
            gbc_sb = keep.tile([BLK, OUT_DIM], F32)
            nc.sync.dma_start(out=gbc_sb, in_=gbc_d)
            bbc_sb = keep.tile([BLK, OUT_DIM], F32)
            nc.sync.dma_start(out=bbc_sb, in_=bbc_d)

            wrhT_sb = keep.tile([128, N], BF16)       # (h@W_r)^T  [hd, j]
            wlhT_sb = keep.tile([128, BLK], F32)      # (hblk@W_l)^T [hd, i]
            vext_sb = keep.tile([128, NJT * (D + 1) * H], BF16)  # V + ones cols
            srT_sb = keep.tile([H, N], BF16)          # sr^T [h, j]
            maskb_sb = keep.tile([BLK, N], BF16)      # (adj-1)*100
            wT_sb = keep.tile([128, NJT * H * BLK], BF16)  # exp scores [j,(i,h)]
            agg_sb = keep.tile([BLK, OUT_DIM], F32)

            with tc.tile_pool(name="ps0", bufs=2, space="PSUM") as ps0:
                # WrhT = W_r^T @ h^T  -> bf16 (gates stage 1)
                for half in range(2):
                    big = ps0.tile([128, 512], F32, tag="big")
                    nc.tensor.matmul(big, wr_sb, hT_sb[:, half * 512:(half + 1) * 512],
                                     start=True, stop=True)
                    nc.vector.tensor_copy(wrhT_sb[:, half * 512:(half + 1) * 512], big)
                # WlhT (this block), kept f32 for scalar/bias use
                wp = ps0.tile([128, 128], F32, tag="tp", bufs=1)
                nc.tensor.matmul(wp, wl_sb, hblkT_sb, start=True, stop=True)
                nc.vector.tensor_copy(wlhT_sb, wp)
                # mask = (adj-1)*100 in bf16 (cheap, DVE, before the absp stream)
                nc.vector.tensor_scalar(maskb_sb, adj_sb, 1.0, 100.0,
                                        ALU.subtract, ALU.mult)
                # srT = a^T . WrhT per head (gates the stage-1 sr matmuls)
                for half in range(2):
                    sp = ps0.tile([H, 512], F32, tag="sr", bufs=1)
                    nc.tensor.matmul(sp, a1_sb,
                                     wrhT_sb[:, half * 512:(half + 1) * 512],
                                     start=True, stop=True)
                    nc.vector.tensor_copy(
                        srT_sb[:, half * 512:(half + 1) * 512], sp)
                # V projection + Vext (ACT evacuates to overlap with DVE work)
                nc.vector.memset(vext_sb, 1.0)
                for jt in range(NJT):
                    vp = ps0.tile([128, 128], F32, tag="vp", bufs=4)
                    nc.tensor.matmul(vp, hT_sb[:, jt * 128:(jt + 1) * 128],
                                     wv_sb, start=True, stop=True)
                    base = jt * (D + 1) * H
                    for hh in range(H):
                        nc.scalar.copy(
                            vext_sb[:, base + hh * (D + 1):
                                    base + hh * (D + 1) + D],
                            vp[:, hh * D:(hh + 1) * D])

            # ------------- stage 1: pairwise scores -------------
            with tc.tile_pool(name="ps1", bufs=NJT, space="PSUM") as ps1, \
                 tc.tile_pool(name="abs", bufs=6) as absp_pool:
                banks = [ps1.tile([128, H * BLK], F32, name=f"bank{jt}", tag="bank")
                         for jt in range(NJT)]
                for jt in range(NJT):
                    # sr term (rank-4) opens each bank's accumulation group
                    nc.tensor.matmul(banks[jt], srT_sb[:, jt * 128:(jt + 1) * 128],
                                     repc_sb, start=True, stop=False,
                                     skip_group_check=True)
                    nc.tensor.matmul(banks[jt], maskb_sb[:, jt * 128:(jt + 1) * 128],
                                     i4rep_sb, start=False, stop=False,
                                     skip_group_check=True)
                for i in range(BLK):
                    absp = absp_pool.tile([128, N], BF16, tag="absp")
                    if _on_act(i):
                        # |WrhT + wl_i|
                        nc.scalar.activation(absp, wrhT_sb, AF.Abs,
                                             bias=wlhT_sb[:, i:i + 1], scale=1.0)
                        arhs = aact_sb
                    else:
                        # min(WrhT + wl_i, 0)
                        nc.vector.tensor_scalar(absp, wrhT_sb,
                                                wlhT_sb[:, i:i + 1],
                                                0.0, ALU.add, ALU.min)
                        arhs = adve_sb
                    for jt in range(NJT):
                        nc.tensor.matmul(banks[jt][:, H * i:H * i + H],
                                         absp[:, jt * 128:(jt + 1) * 128], arhs,
                                         start=False, stop=(i == BLK - 1),
                                         skip_group_check=True)
                # exp -> w^T bf16 (bank_jt frees as soon as its exp is done)
                for jt in range(NJT):
                    nc.scalar.activation(wT_sb[:, jt * 512:(jt + 1) * 512],
                                         banks[jt], AF.Exp)

            # ------------- stage 3: aggregate, pipelined with exp -------------
            with tc.tile_pool(name="ps3", bufs=4, space="PSUM") as ps3:
                accs = [ps3.tile([BLK, D + 1], F32, name=f"acc{hh}", tag="acc") for hh in range(H)]
                for jt in range(NJT):
                    for hh in range(H):
                        lhsT = wT_sb[:, jt * 512 + hh:(jt + 1) * 512: H].opt()
                        rhs = vext_sb[:, jt * (D + 1) * H + hh * (D + 1):
                                      jt * (D + 1) * H + (hh + 1) * (D + 1)]
                        nc.tensor.matmul(accs[hh], lhsT, rhs,
                                         start=(jt == 0), stop=(jt == NJT - 1),
                                         skip_group_check=True)
                for hh in range(H):
                    rinv = small.tile([BLK, 1], F32, tag="rinv")
                    nc.vector.reciprocal(rinv, accs[hh][:, D:D + 1])
                    nc.vector.tensor_scalar_mul(
                        agg_sb[:, hh * D:(hh + 1) * D], accs[hh][:, 0:D], rinv)

            # ---------------- stage 4: LayerNorm + ReLU ----------------
            stats = small.tile([BLK, 6], F32, tag="stats")
            nc.vector.bn_stats(out=stats, in_=agg_sb)
            mv = small.tile([BLK, 2], F32, tag="mv")
            nc.vector.bn_aggr(out=mv, in_=stats)
            cent = keep.tile([BLK, OUT_DIM], F32)
            nc.vector.tensor_scalar_sub(cent, agg_sb, mv[:, 0:1])
            epsb = small.tile([BLK, 1], F32, tag="epsb")
            nc.vector.memset(epsb, 1e-5)
            lnv = small.tile([BLK, 1], F32, tag="lnv")
            nc.scalar.activation(lnv, mv[:, 1:2], AF.Ln, bias=epsb, scale=1.0)
            rstd = small.tile([BLK, 1], F32, tag="rstd")
            nc.scalar.activation(rstd, lnv, AF.Exp, scale=-0.5)
            yt = keep.tile([BLK, OUT_DIM], F32)
            nc.vector.tensor_scalar_mul(yt, cent, rstd)
            nc.vector.tensor_tensor(yt, yt, gbc_sb, ALU.mult)
            nc.vector.tensor_tensor(yt, yt, bbc_sb, ALU.add)
            nc.vector.tensor_scalar_max(yt, yt, 0.0)
            nc.sync.dma_start(out=y_d, in_=yt)

    nc.compile()
    return nc


_NC = None


def _get_program():
    global _NC
    if _NC is None:
        _NC = build_program()
    return _NC


def _consts(a, ln_g, ln_b):
    bf = ml_dtypes.bfloat16
    a = np.asarray(a, np.float32)
    Adve = np.zeros((128, H), np.float32)
    Aact = np.zeros((128, H), np.float32)
    A1 = np.zeros((128, H), np.float32)
    for hh in range(H):
        Adve[hh * D:(hh + 1) * D, hh] = -0.8 * a
        Aact[hh * D:(hh + 1) * D, hh] = 0.4 * a
        A1[hh * D:(hh + 1) * D, hh] = a
    # per-i sr coefficient: 0.6 for ACT(|.|) rows, 1.0 for DVE(min) rows
    rep_c = np.zeros((H, H * BLK), np.float32)
    for i in range(BLK):
        c = 0.6 if _on_act(i) else 1.0
        rep_c[:, H * i:H * i + H] = c * np.eye(H, dtype=np.float32)
    I4rep = np.repeat(np.eye(BLK, dtype=np.float32), H, axis=1)
    return {
        "Adve": Adve.astype(bf), "Aact": Aact.astype(bf), "A1": A1.astype(bf),
        "rep_c": rep_c.astype(bf), "I4rep": I4rep.astype(bf),
        "gbc": np.tile(np.asarray(ln_g, np.float32)[None, :], (BLK, 1)),
        "bbc": np.tile(np.asarray(ln_b, np.float32)[None, :], (BLK, 1)),
    }


def kernel(h, adj, W_l, W_r, W_v, a, ln_g, ln_b, _trace=False, _tmpdir=None):
    nc = _get_program()
    h = np.asarray(h, np.float32)
    hT = np.ascontiguousarray(h.T)
    adj = np.ascontiguousarray(np.asarray(adj, np.int32))
    consts = _consts(a, ln_g, ln_b)
    base = {
        "hT": hT,
        "W_l": np.ascontiguousarray(np.asarray(W_l, np.float32)),
        "W_r": np.ascontiguousarray(np.asarray(W_r, np.float32)),
        "W_v": np.ascontiguousarray(np.asarray(W_v, np.float32)),
        **consts,
    }
    in_maps = []
    for c in range(NCORES):
        m = dict(base)
        m["hblkT"] = np.ascontiguousarray(hT[:, c * BLK:(c + 1) * BLK])
        m["adjrow"] = adj[c * BLK:(c + 1) * BLK]
        in_maps.append(m)
    kw = {}
    if _trace:
        kw = dict(trace=True, tmpdir=_tmpdir)
    res = run_bass_kernel_spmd(nc, in_maps, list(range(NCORES)), **kw)
    y = np.concatenate([res.results[c]["y"] for c in range(NCORES)], axis=0)
    if _trace:
        return y, res
    return y


# revision 12
# speedup vs baseline: 1.0768x; 1.0527x over previous
"""GATv2 layer (N=1024, IN=OUT=128, H=4, D=32) on 8 Trainium2 NeuronCores.

Sharding: row-block of the output/adjacency (128 rows of i per core);
node features (pre-transposed h^T) and the projection weights are replicated.

Math per core (rows i of this core's block), with leakyrelu(x) = x - 0.8*min(x,0)
= 0.6*x + 0.4*|x| and sl[i,h] = a.Wlh[i,h,:] (cancels in the softmax over j),
sr[j,h] = a.Wrh[j,h,:]:

  e[i,j,h] = c*sr[j,h] + m_i[:,j] @ blockdiag(s*a)  - 100*(1-adj[i,j])

where per i-row either m_i = min(Wrh^T + Wlh_i, 0) produced on DVE
(tensor_scalar add+min, c=1, s=-0.8) or m_i = |Wrh^T + Wlh_i| produced on ACT
(Abs with per-partition bias, c=0.6, s=0.4).  Scores are O(3) so no
max-subtraction is needed, and the -100 mask term underflows exp to exact 0.
The d-reduction runs on the PE with m_i as the (FWL bf16) weights:
  bank_jt[j_local, 4*i+h] += m_i[:, jt].T @ blockdiag(s*a)
Two more accumulating matmuls per bank add c*sr (rank-4, with the per-column c
baked into the rhs) and the mask (via an identity-expansion rhs).  ACT
exponentiates banks into bf16 w^T tiles [j, (i,h)] and the PE aggregates
agg[i,(h,d)] plus the softmax denominators in one pass using V extended with a
ones-column.  LayerNorm: bn_stats + rstd = exp(-0.5*ln(var+eps)).
"""
import numpy as np
import ml_dtypes

import concourse.bacc as bacc
import concourse.tile as tile
from concourse import mybir
from concourse.bass_utils import run_bass_kernel_spmd

N = 1024
IN_DIM = 128
OUT_DIM = 128
H = 4
D = 32
NCORES = 8
BLK = N // NCORES  # 128 rows of i per core
NJT = 8            # j tiles of 128
F32 = mybir.dt.float32
BF16 = mybir.dt.bfloat16
I32 = mybir.dt.int32
AF = mybir.ActivationFunctionType
ALU = mybir.AluOpType

ACT_EVERY = 4  # every ACT_EVERY-th |pair| tile is produced on ACT


def _on_act(i):
    return i % ACT_EVERY == ACT_EVERY - 1


def build_program():
    nc = bacc.Bacc(trn_type="TRN2", target_bir_lowering=False, debug=False,
                   num_devices=NCORES)

    def din(name, shape, dt):
        return nc.dram_tensor(name, shape, dt, kind="ExternalInput").ap()

    hT_d = din("hT", [IN_DIM, N], F32)               # h^T (host-transposed)
    hblkT_d = din("hblkT", [IN_DIM, BLK], F32)       # this core's rows of h, ^T
    adj_d = din("adjrow", [BLK, N], I32)             # this core's adj rows
    wl_d = din("W_l", [IN_DIM, OUT_DIM], F32)
    wr_d = din("W_r", [IN_DIM, OUT_DIM], F32)
    wv_d = din("W_v", [IN_DIM, OUT_DIM], F32)
    adve_d = din("Adve", [128, H], BF16)             # blockdiag(-0.8*a)
    aact_d = din("Aact", [128, H], BF16)             # blockdiag(+0.4*a)
    a1_d = din("A1", [128, H], BF16)                 # blockdiag(a)
    repc_d = din("rep_c", [H, H * BLK], BF16)        # c_i * I4 per i-column
    i4rep_d = din("I4rep", [BLK, H * BLK], BF16)     # repeat(I128, 4, axis=1)
    gbc_d = din("gbc", [BLK, OUT_DIM], F32)          # ln_g broadcast
    bbc_d = din("bbc", [BLK, OUT_DIM], F32)          # ln_b broadcast
    y_d = nc.dram_tensor("y", [BLK, OUT_DIM], F32, kind="ExternalOutput").ap()

    with tile.TileContext(nc) as tc:
        with tc.tile_pool(name="keep", bufs=1) as keep, \
             tc.tile_pool(name="small", bufs=4) as small:
            # --- loads on the stage-1 critical path first ---
            hT_sb = keep.tile([IN_DIM, N], F32)
            nc.sync.dma_start(out=hT_sb, in_=hT_d)
            wr_sb = keep.tile([IN_DIM, OUT_DIM], F32)
            nc.sync.dma_start(out=wr_sb, in_=wr_d)
            hblkT_sb = keep.tile([IN_DIM, BLK], F32)
            nc.sync.dma_start(out=hblkT_sb, in_=hblkT_d)
            wl_sb = keep.tile([IN_DIM, OUT_DIM], F32)
            nc.sync.dma_start(out=wl_sb, in_=wl_d)
            adve_sb = keep.tile([128, H], BF16)
            nc.sync.dma_start(out=adve_sb, in_=adve_d)
            aact_sb = keep.tile([128, H], BF16)
            nc.sync.dma_start(out=aact_sb, in_=aact_d)
            adj_sb = keep.tile([BLK, N], I32)
            nc.gpsimd.dma_start(out=adj_sb, in_=adj_d)
            wv_sb = keep.tile([IN_DIM, OUT_DIM], F32)
            nc.gpsimd.dma_start(out=wv_sb, in_=wv_d)
            a1_sb = keep.tile([128, H], BF16)
            nc.sync.dma_start(out=a1_sb, in_=a1_d)
            repc_sb = keep.tile([H, H * BLK], BF16)
            nc.gpsimd.dma_start(out=repc_sb, in_=repc_d)
            i4rep_sb = keep.tile([BLK, H * BLK], BF16)
            nc.gpsimd.dma_start(out=i4rep_sb, in_=i4rep_d)
            gbc_sb = keep.tile([BLK, OUT_DIM], F32)
            nc.gpsimd.dma_start(out=gbc_sb, in_=gbc_d)
            bbc_sb = keep.tile([BLK, OUT_DIM], F32)
            nc.gpsimd.dma_start(out=bbc_sb, in_=bbc_d)

            tldummy = small.tile([BLK, 1], F32, tag="tldummy")
            nc.vector.memset(tldummy, 1.0)
            nc.scalar.activation(tldummy, tldummy, AF.Ln)
            wrhT_sb = keep.tile([128, N], BF16)       # (h@W_r)^T  [hd, j]
            wlhT_sb = keep.tile([128, BLK], F32)      # (hblk@W_l)^T [hd, i]
            vext_sb = keep.tile([128, NJT * (D + 1) * H], BF16)  # V + ones cols
            srT_sb = keep.tile([H, N], BF16)          # sr^T [h, j]
            maskb_sb = keep.tile([BLK, N], BF16)      # (adj-1)*100
            wT_sb = keep.tile([128, NJT * H * BLK], BF16)  # exp scores [j,(i,h)]
            agg_sb = keep.tile([BLK, OUT_DIM], F32)

            with tc.tile_pool(name="ps0", bufs=2, space="PSUM") as ps0:
                # WrhT = W_r^T @ h^T  -> bf16 (gates stage 1)
                for half in range(2):
                    big = ps0.tile([128, 512], F32, tag="big")
                    nc.tensor.matmul(big, wr_sb, hT_sb[:, half * 512:(half + 1) * 512],
                                     start=True, stop=True)
                    nc.vector.tensor_copy(wrhT_sb[:, half * 512:(half + 1) * 512], big)
                # WlhT (this block), kept f32 for scalar/bias use
                wp = ps0.tile([128, 128], F32, tag="tp", bufs=1)
                nc.tensor.matmul(wp, wl_sb, hblkT_sb, start=True, stop=True)
                nc.vector.tensor_copy(wlhT_sb, wp)
                # srT = a^T . WrhT per head
                for half in range(2):
                    sp = ps0.tile([H, 512], F32, tag="sr", bufs=1)
                    nc.tensor.matmul(sp, a1_sb,
                                     wrhT_sb[:, half * 512:(half + 1) * 512],
                                     start=True, stop=True)
                    nc.vector.tensor_copy(
                        srT_sb[:, half * 512:(half + 1) * 512], sp)
                # V projection + Vext (ACT evacuates to overlap with DVE work)
                nc.vector.memset(vext_sb, 1.0)
                for jt in range(NJT):
                    vp = ps0.tile([128, 128], F32, tag="vp", bufs=4)
                    nc.tensor.matmul(vp, hT_sb[:, jt * 128:(jt + 1) * 128],
                                     wv_sb, start=True, stop=True)
                    base = jt * (D + 1) * H
                    for hh in range(H):
                        nc.scalar.copy(
                            vext_sb[:, base + hh * (D + 1):
                                    base + hh * (D + 1) + D],
                            vp[:, hh * D:(hh + 1) * D])

            # ------------- stage 1: pairwise scores -------------
            with tc.tile_pool(name="ps1", bufs=NJT, space="PSUM") as ps1, \
                 tc.tile_pool(name="abs", bufs=6) as absp_pool:
                banks = [ps1.tile([128, H * BLK], F32, name=f"bank{jt}", tag="bank")
                         for jt in range(NJT)]
                for jt in range(NJT):
                    # sr term (rank-4, full bank) opens each accumulation group
                    nc.tensor.matmul(banks[jt], srT_sb[:, jt * 128:(jt + 1) * 128],
                                     repc_sb, start=True, stop=False,
                                     skip_group_check=True)
                for i in range(BLK):
                    absp = absp_pool.tile([128, N], BF16, tag="absp")
                    if _on_act(i):
                        # |WrhT + wl_i|
                        nc.scalar.activation(absp, wrhT_sb, AF.Abs,
                                             bias=wlhT_sb[:, i:i + 1], scale=1.0)
                        arhs = aact_sb
                    else:
                        # min(WrhT + wl_i, 0)
                        nc.vector.tensor_scalar(absp, wrhT_sb,
                                                wlhT_sb[:, i:i + 1],
                                                0.0, ALU.add, ALU.min)
                        arhs = adve_sb
                    for jt in range(NJT):
                        nc.tensor.matmul(banks[jt][:, H * i:H * i + H],
                                         absp[:, jt * 128:(jt + 1) * 128], arhs,
                                         start=False, stop=False,
                                         skip_group_check=True)
                    if i == BLK // 2:
                        # mask needs only the adj DMA; computed mid-stream so
                        # it never gates the start of stage 1
                        nc.vector.tensor_scalar(maskb_sb, adj_sb, 1.0, 100.0,
                                                ALU.subtract, ALU.mult)
                # mask term accumulates last (order within a sum is free)
                for jt in range(NJT):
                    nc.tensor.matmul(banks[jt], maskb_sb[:, jt * 128:(jt + 1) * 128],
                                     i4rep_sb, start=False, stop=True,
                                     skip_group_check=True)
                # exp -> w^T bf16 (bank_jt frees as soon as its exp is done)
                for jt in range(NJT):
                    nc.scalar.activation(wT_sb[:, jt * 512:(jt + 1) * 512],
                                         banks[jt], AF.Exp)

            # ------------- stage 3: aggregate, pipelined with exp -------------
            with tc.tile_pool(name="ps3", bufs=4, space="PSUM") as ps3:
                accs = [ps3.tile([BLK, D + 1], F32, name=f"acc{hh}", tag="acc") for hh in range(H)]
                for jt in range(NJT):
                    for hh in range(H):
                        lhsT = wT_sb[:, jt * 512 + hh:(jt + 1) * 512: H].opt()
                        rhs = vext_sb[:, jt * (D + 1) * H + hh * (D + 1):
                                      jt * (D + 1) * H + (hh + 1) * (D + 1)]
                        nc.tensor.matmul(accs[hh], lhsT, rhs,
                                         start=(jt == 0), stop=(jt == NJT - 1),
                                         skip_group_check=True)
                for hh in range(H):
                    rinv = small.tile([BLK, 1], F32, tag="rinv")
                    nc.vector.reciprocal(rinv, accs[hh][:, D:D + 1])
                    nc.vector.tensor_scalar_mul(
                        agg_sb[:, hh * D:(hh + 1) * D], accs[hh][:, 0:D], rinv)

            # ---------------- stage 4: LayerNorm + ReLU ----------------
            stats = small.tile([BLK, 6], F32, tag="stats")
            nc.vector.bn_stats(out=stats, in_=agg_sb)
            mv = small.tile([BLK, 2], F32, tag="mv")
            nc.vector.bn_aggr(out=mv, in_=stats)
            cent = keep.tile([BLK, OUT_DIM], F32)
            nc.vector.tensor_scalar_sub(cent, agg_sb, mv[:, 0:1])
            epsb = small.tile([BLK, 1], F32, tag="epsb")
            nc.vector.memset(epsb, 1e-5)
            lnv = small.tile([BLK, 1], F32, tag="lnv")
            nc.scalar.activation(lnv, mv[:, 1:2], AF.Ln, bias=epsb, scale=1.0)
            rstd = small.tile([BLK, 1], F32, tag="rstd")
            nc.scalar.activation(rstd, lnv, AF.Exp, scale=-0.5)
            yt = keep.tile([BLK, OUT_DIM], F32)
            nc.vector.tensor_scalar_mul(yt, cent, rstd)
            nc.vector.tensor_tensor(yt, yt, gbc_sb, ALU.mult)
            nc.vector.tensor_tensor(yt, yt, bbc_sb, ALU.add)
            nc.vector.tensor_scalar_max(yt, yt, 0.0)
            nc.sync.dma_start(out=y_d, in_=yt)

    nc.compile()
    return nc


_NC = None


def _get_program():
    global _NC
    if _NC is None:
        _NC = build_program()
    return _NC


def _consts(a, ln_g, ln_b):
    bf = ml_dtypes.bfloat16
    a = np.asarray(a, np.float32)
    Adve = np.zeros((128, H), np.float32)
    Aact = np.zeros((128, H), np.float32)
    A1 = np.zeros((128, H), np.float32)
    for hh in range(H):
        Adve[hh * D:(hh + 1) * D, hh] = -0.8 * a
        Aact[hh * D:(hh + 1) * D, hh] = 0.4 * a
        A1[hh * D:(hh + 1) * D, hh] = a
    # per-i sr coefficient: 0.6 for ACT(|.|) rows, 1.0 for DVE(min) rows
    rep_c = np.zeros((H, H * BLK), np.float32)
    for i in range(BLK):
        c = 0.6 if _on_act(i) else 1.0
        rep_c[:, H * i:H * i + H] = c * np.eye(H, dtype=np.float32)
    I4rep = np.repeat(np.eye(BLK, dtype=np.float32), H, axis=1)
    return {
        "Adve": Adve.astype(bf), "Aact": Aact.astype(bf), "A1": A1.astype(bf),
        "rep_c": rep_c.astype(bf), "I4rep": I4rep.astype(bf),
        "gbc": np.tile(np.asarray(ln_g, np.float32)[None, :], (BLK, 1)),
        "bbc": np.tile(np.asarray(ln_b, np.float32)[None, :], (BLK, 1)),
    }


def kernel(h, adj, W_l, W_r, W_v, a, ln_g, ln_b, _trace=False, _tmpdir=None):
    nc = _get_program()
    h = np.asarray(h, np.float32)
    hT = np.ascontiguousarray(h.T)
    adj = np.ascontiguousarray(np.asarray(adj, np.int32))
    consts = _consts(a, ln_g, ln_b)
    base = {
        "hT": hT,
        "W_l": np.ascontiguousarray(np.asarray(W_l, np.float32)),
        "W_r": np.ascontiguousarray(np.asarray(W_r, np.float32)),
        "W_v": np.ascontiguousarray(np.asarray(W_v, np.float32)),
        **consts,
    }
    in_maps = []
    for c in range(NCORES):
        m = dict(base)
        m["hblkT"] = np.ascontiguousarray(hT[:, c * BLK:(c + 1) * BLK])
        m["adjrow"] = adj[c * BLK:(c + 1) * BLK]
        in_maps.append(m)
    kw = {}
    if _trace:
        kw = dict(trace=True, tmpdir=_tmpdir)
    res = run_bass_kernel_spmd(nc, in_maps, list(range(NCORES)), **kw)
    y = np.concatenate([res.results[c]["y"] for c in range(NCORES)], axis=0)
    if _trace:
        return y, res
    return y


# revision 14
# speedup vs baseline: 1.0868x; 1.0093x over previous
"""GATv2 layer (N=1024, IN=OUT=128, H=4, D=32) on 8 Trainium2 NeuronCores.

Sharding: row-block of the output/adjacency (128 rows of i per core);
node features (pre-transposed h^T) and the projection weights are replicated.

Math per core (rows i of this core's block), with leakyrelu(x) = x - 0.8*min(x,0)
= 0.6*x + 0.4*|x| and sl[i,h] = a.Wlh[i,h,:] (cancels in the softmax over j),
sr[j,h] = a.Wrh[j,h,:]:

  e[i,j,h] = c*sr[j,h] + m_i[:,j] @ blockdiag(s*a)  - 100*(1-adj[i,j])

where per i-row either m_i = min(Wrh^T + Wlh_i, 0) produced on DVE
(tensor_scalar add+min, c=1, s=-0.8) or m_i = |Wrh^T + Wlh_i| produced on ACT
(Abs with per-partition bias, c=0.6, s=0.4).  Scores are O(3) so no
max-subtraction is needed, and the -100 mask term underflows exp to exact 0.
The d-reduction runs on the PE with m_i as the (FWL bf16) weights:
  bank_jt[j_local, 4*i+h] += m_i[:, jt].T @ blockdiag(s*a)
Two more accumulating matmuls per bank add c*sr (rank-4, with the per-column c
baked into the rhs) and the mask (via an identity-expansion rhs).  ACT
exponentiates banks into bf16 w^T tiles [j, (i,h)] and the PE aggregates
agg[i,(h,d)] plus the softmax denominators in one pass using V extended with a
ones-column.  LayerNorm: bn_stats + rstd = exp(-0.5*ln(var+eps)).
"""
import numpy as np
import ml_dtypes

import concourse.bacc as bacc
import concourse.tile as tile
from concourse import mybir
from concourse.bass_utils import run_bass_kernel_spmd

N = 1024
IN_DIM = 128
OUT_DIM = 128
H = 4
D = 32
NCORES = 8
BLK = N // NCORES  # 128 rows of i per core
NJT = 8            # j tiles of 128
F32 = mybir.dt.float32
BF16 = mybir.dt.bfloat16
I32 = mybir.dt.int32
AF = mybir.ActivationFunctionType
ALU = mybir.AluOpType

ACT_EVERY = 4  # every ACT_EVERY-th |pair| tile is produced on ACT


def _on_act(i):
    return i % ACT_EVERY == ACT_EVERY - 1


def build_program():
    nc = bacc.Bacc(trn_type="TRN2", target_bir_lowering=False, debug=False,
                   num_devices=NCORES)

    def din(name, shape, dt):
        return nc.dram_tensor(name, shape, dt, kind="ExternalInput").ap()

    # packed critical f32 inputs: hT | hblkT | W_r | W_l
    critf_d = din("critf", [128, N + BLK + 2 * OUT_DIM], F32)
    critb_d = din("critb", [128, 3 * H], BF16)       # Adve | Aact | A1
    adj_d = din("adjrow", [BLK, N], I32)             # this core's adj rows
    miscf_d = din("miscf", [128, OUT_DIM * 3], F32)  # W_v | gbc | bbc
    repc_d = din("rep_c", [H, H * BLK], BF16)        # c_i * I4 per i-column
    i4rep_d = din("I4rep", [BLK, H * BLK], BF16)     # repeat(I128, 4, axis=1)
    y_d = nc.dram_tensor("y", [BLK, OUT_DIM], F32, kind="ExternalOutput").ap()

    with tile.TileContext(nc) as tc:
        with tc.tile_pool(name="keep", bufs=1) as keep, \
             tc.tile_pool(name="small", bufs=4) as small:
            # --- loads: one packed DMA on the critical path ---
            critf_sb = keep.tile([128, N + BLK + 2 * OUT_DIM], F32)
            nc.sync.dma_start(out=critf_sb, in_=critf_d)
            critb_sb = keep.tile([128, 3 * H], BF16)
            nc.sync.dma_start(out=critb_sb, in_=critb_d)
            hT_sb = critf_sb[:, 0:N]
            hblkT_sb = critf_sb[:, N:N + BLK]
            wr_sb = critf_sb[:, N + BLK:N + BLK + OUT_DIM]
            wl_sb = critf_sb[:, N + BLK + OUT_DIM:N + BLK + 2 * OUT_DIM]
            adve_sb = critb_sb[:, 0:H]
            aact_sb = critb_sb[:, H:2 * H]
            a1_sb = critb_sb[:, 2 * H:3 * H]
            adj_sb = keep.tile([BLK, N], I32)
            nc.gpsimd.dma_start(out=adj_sb, in_=adj_d)
            miscf_sb = keep.tile([128, OUT_DIM * 3], F32)
            nc.gpsimd.dma_start(out=miscf_sb, in_=miscf_d)
            wv_sb = miscf_sb[:, 0:OUT_DIM]
            gbc_sb = miscf_sb[:, OUT_DIM:2 * OUT_DIM]
            bbc_sb = miscf_sb[:, 2 * OUT_DIM:3 * OUT_DIM]
            repc_sb = keep.tile([H, H * BLK], BF16)
            nc.gpsimd.dma_start(out=repc_sb, in_=repc_d)
            i4rep_sb = keep.tile([BLK, H * BLK], BF16)
            nc.gpsimd.dma_start(out=i4rep_sb, in_=i4rep_d)

            wrhT_sb = keep.tile([128, N], BF16)       # (h@W_r)^T  [hd, j]
            wlhT_sb = keep.tile([128, BLK], F32)      # (hblk@W_l)^T [hd, i]
            vext_sb = keep.tile([128, NJT * (D + 1) * H], BF16)  # V + ones cols
            srT_sb = keep.tile([H, N], BF16)          # sr^T [h, j]
            maskb_sb = keep.tile([BLK, N], BF16)      # (adj-1)*100
            wT_sb = keep.tile([128, NJT * H * BLK], BF16)  # exp scores [j,(i,h)]
            agg_sb = keep.tile([BLK, OUT_DIM], F32)

            with tc.tile_pool(name="ps0", bufs=2, space="PSUM") as ps0:
                # WrhT = W_r^T @ h^T  -> bf16 (gates stage 1)
                for half in range(2):
                    big = ps0.tile([128, 512], F32, tag="big")
                    nc.tensor.matmul(big, wr_sb, hT_sb[:, half * 512:(half + 1) * 512],
                                     start=True, stop=True)
                    nc.vector.tensor_copy(wrhT_sb[:, half * 512:(half + 1) * 512], big)
                # WlhT (this block), kept f32 for scalar/bias use
                wp = ps0.tile([128, 128], F32, tag="tp", bufs=1)
                nc.tensor.matmul(wp, wl_sb, hblkT_sb, start=True, stop=True)
                nc.vector.tensor_copy(wlhT_sb, wp)
                # srT = a^T . WrhT per head
                for half in range(2):
                    sp = ps0.tile([H, 512], F32, tag="sr", bufs=1)
                    nc.tensor.matmul(sp, a1_sb,
                                     wrhT_sb[:, half * 512:(half + 1) * 512],
                                     start=True, stop=True)
                    nc.vector.tensor_copy(
                        srT_sb[:, half * 512:(half + 1) * 512], sp)
                # V projection + Vext (ACT evacuates to overlap with DVE work)
                nc.vector.memset(vext_sb, 1.0)
                for jt in range(NJT):
                    vp = ps0.tile([128, 128], F32, tag="vp", bufs=4)
                    nc.tensor.matmul(vp, hT_sb[:, jt * 128:(jt + 1) * 128],
                                     wv_sb, start=True, stop=True)
                    base = jt * (D + 1) * H
                    for hh in range(H):
                        nc.scalar.copy(
                            vext_sb[:, base + hh * (D + 1):
                                    base + hh * (D + 1) + D],
                            vp[:, hh * D:(hh + 1) * D])

            # ------------- stage 1: pairwise scores -------------
            with tc.tile_pool(name="ps1", bufs=NJT, space="PSUM") as ps1, \
                 tc.tile_pool(name="abs", bufs=6) as absp_pool:
                banks = [ps1.tile([128, H * BLK], F32, name=f"bank{jt}", tag="bank")
                         for jt in range(NJT)]
                for jt in range(NJT):
                    # sr term (rank-4, full bank) opens each accumulation group
                    nc.tensor.matmul(banks[jt], srT_sb[:, jt * 128:(jt + 1) * 128],
                                     repc_sb, start=True, stop=False,
                                     skip_group_check=True)
                for i in range(BLK):
                    absp = absp_pool.tile([128, N], BF16, tag="absp")
                    if _on_act(i):
                        # |WrhT + wl_i|
                        nc.scalar.activation(absp, wrhT_sb, AF.Abs,
                                             bias=wlhT_sb[:, i:i + 1], scale=1.0)
                        arhs = aact_sb
                    else:
                        # min(WrhT + wl_i, 0)
                        nc.vector.tensor_scalar(absp, wrhT_sb,
                                                wlhT_sb[:, i:i + 1],
                                                0.0, ALU.add, ALU.min)
                        arhs = adve_sb
                    for jt in range(NJT):
                        nc.tensor.matmul(banks[jt][:, H * i:H * i + H],
                                         absp[:, jt * 128:(jt + 1) * 128], arhs,
                                         start=False, stop=False,
                                         skip_group_check=True)
                    if i == BLK // 2:
                        # mask needs only the adj DMA; computed mid-stream so
                        # it never gates the start of stage 1
                        nc.vector.tensor_scalar(maskb_sb, adj_sb, 1.0, 100.0,
                                                ALU.subtract, ALU.mult)
                # mask term accumulates last (order within a sum is free)
                for jt in range(NJT):
                    nc.tensor.matmul(banks[jt], maskb_sb[:, jt * 128:(jt + 1) * 128],
                                     i4rep_sb, start=False, stop=True,
                                     skip_group_check=True)
                # exp -> w^T bf16 (bank_jt frees as soon as its exp is done)
                for jt in range(NJT):
                    nc.scalar.activation(wT_sb[:, jt * 512:(jt + 1) * 512],
                                         banks[jt], AF.Exp)

            # ------------- stage 3: aggregate, pipelined with exp -------------
            with tc.tile_pool(name="ps3", bufs=4, space="PSUM") as ps3:
                accs = [ps3.tile([BLK, D + 1], F32, name=f"acc{hh}", tag="acc") for hh in range(H)]
                for jt in range(NJT):
                    for hh in range(H):
                        lhsT = wT_sb[:, jt * 512 + hh:(jt + 1) * 512: H].opt()
                        rhs = vext_sb[:, jt * (D + 1) * H + hh * (D + 1):
                                      jt * (D + 1) * H + (hh + 1) * (D + 1)]
                        nc.tensor.matmul(accs[hh], lhsT, rhs,
                                         start=(jt == 0), stop=(jt == NJT - 1),
                                         skip_group_check=True)
                for hh in range(H):
                    rinv = small.tile([BLK, 1], F32, tag="rinv")
                    nc.vector.reciprocal(rinv, accs[hh][:, D:D + 1])
                    nc.vector.tensor_scalar_mul(
                        agg_sb[:, hh * D:(hh + 1) * D], accs[hh][:, 0:D], rinv)

            # ---------------- stage 4: LayerNorm + ReLU ----------------
            stats = small.tile([BLK, 6], F32, tag="stats")
            nc.vector.bn_stats(out=stats, in_=agg_sb)
            mv = small.tile([BLK, 2], F32, tag="mv")
            nc.vector.bn_aggr(out=mv, in_=stats)
            cent = keep.tile([BLK, OUT_DIM], F32)
            nc.vector.tensor_scalar_sub(cent, agg_sb, mv[:, 0:1])
            # rstd = 1/sqrt(var+eps): Quake initial guess + 2 Newton steps (DVE)
            veps = small.tile([BLK, 1], F32, tag="veps")
            nc.vector.tensor_scalar_add(veps, mv[:, 1:2], 1e-5)
            rstd = small.tile([BLK, 1], F32, tag="rstd")
            nc.vector.tensor_scalar(rstd.bitcast(I32), veps.bitcast(I32), 1,
                                    None, ALU.arith_shift_right)
            nc.vector.tensor_scalar(rstd.bitcast(I32), rstd.bitcast(I32), -1,
                                    0x5f3759df, ALU.mult, ALU.add)
            hv = small.tile([BLK, 1], F32, tag="hv")
            nc.vector.tensor_scalar_mul(hv, veps, -0.5)
            for _ in range(2):
                yy = small.tile([BLK, 1], F32, tag="yy")
                nc.vector.tensor_tensor(yy, rstd, rstd, ALU.mult)
                nc.vector.tensor_tensor(yy, yy, hv, ALU.mult)
                nc.vector.tensor_scalar_add(yy, yy, 1.5)
                nc.vector.tensor_tensor(rstd, rstd, yy, ALU.mult)
            yt = keep.tile([BLK, OUT_DIM], F32)
            nc.vector.tensor_scalar_mul(yt, cent, rstd)
            nc.vector.tensor_tensor(yt, yt, gbc_sb, ALU.mult)
            nc.vector.tensor_tensor(yt, yt, bbc_sb, ALU.add)
            nc.vector.tensor_scalar_max(yt, yt, 0.0)
            nc.sync.dma_start(out=y_d, in_=yt)

    nc.compile()
    return nc


_NC = None


def _get_program():
    global _NC
    if _NC is None:
        _NC = build_program()
    return _NC


def _consts(a, ln_g, ln_b):
    bf = ml_dtypes.bfloat16
    a = np.asarray(a, np.float32)
    Adve = np.zeros((128, H), np.float32)
    Aact = np.zeros((128, H), np.float32)
    A1 = np.zeros((128, H), np.float32)
    for hh in range(H):
        Adve[hh * D:(hh + 1) * D, hh] = -0.8 * a
        Aact[hh * D:(hh + 1) * D, hh] = 0.4 * a
        A1[hh * D:(hh + 1) * D, hh] = a
    # per-i sr coefficient: 0.6 for ACT(|.|) rows, 1.0 for DVE(min) rows
    rep_c = np.zeros((H, H * BLK), np.float32)
    for i in range(BLK):
        c = 0.6 if _on_act(i) else 1.0
        rep_c[:, H * i:H * i + H] = c * np.eye(H, dtype=np.float32)
    I4rep = np.repeat(np.eye(BLK, dtype=np.float32), H, axis=1)
    return {
        "Adve": Adve.astype(bf), "Aact": Aact.astype(bf), "A1": A1.astype(bf),
        "rep_c": rep_c.astype(bf), "I4rep": I4rep.astype(bf),
        "gbc": np.tile(np.asarray(ln_g, np.float32)[None, :], (BLK, 1)),
        "bbc": np.tile(np.asarray(ln_b, np.float32)[None, :], (BLK, 1)),
    }


def kernel(h, adj, W_l, W_r, W_v, a, ln_g, ln_b, _trace=False, _tmpdir=None):
    nc = _get_program()
    h = np.asarray(h, np.float32)
    hT = np.ascontiguousarray(h.T)
    adj = np.ascontiguousarray(np.asarray(adj, np.int32))
    consts = _consts(a, ln_g, ln_b)
    W_l = np.asarray(W_l, np.float32)
    W_r = np.asarray(W_r, np.float32)
    W_v = np.asarray(W_v, np.float32)
    critb = np.concatenate([consts["Adve"], consts["Aact"], consts["A1"]], axis=1)
    miscf = np.ascontiguousarray(
        np.concatenate([W_v, consts["gbc"], consts["bbc"]], axis=1))
    base = {
        "critb": np.ascontiguousarray(critb),
        "miscf": miscf,
        "rep_c": consts["rep_c"],
        "I4rep": consts["I4rep"],
    }
    in_maps = []
    for c in range(NCORES):
        m = dict(base)
        m["critf"] = np.ascontiguousarray(np.concatenate(
            [hT, hT[:, c * BLK:(c + 1) * BLK], W_r, W_l], axis=1))
        m["adjrow"] = adj[c * BLK:(c + 1) * BLK]
        in_maps.append(m)
    kw = {}
    if _trace:
        kw = dict(trace=True, tmpdir=_tmpdir)
    res = run_bass_kernel_spmd(nc, in_maps, list(range(NCORES)), **kw)
    y = np.concatenate([res.results[c]["y"] for c in range(NCORES)], axis=0)
    if _trace:
        return y, res
    return y
